# revision 2
# baseline (speedup 1.0000x reference)
"""Trainium2 Bass kernel for differentiable KDE (Gaussian kernel density estimate).

Math (h = 1):
    density[i] = exp(-C||x_i||^2 - ln M) * sum_j w_j * exp(2C x_i.d_j),
                 w_j = exp(-C||d_j||^2),  C = 0.5/sqrt(2*pi)

Sharding: data-parallel over x rows (1024 per core), data replicated.

v5 design notes:
  - ACT (scalar engine) is the hard bottleneck: exp runs at 1
    elem/lane/cycle + ~350 cycles/instruction. The d-norm factor w_j is
    applied as the stationary weight of the reduction matvec (not an ACT
    bias), so activations are biasless and one ACTIVATE spans 3 j-tiles
    (FD=1536). The exp table set is preloaded by a dummy activation at
    t=0 so the first real exp doesn't pay the ~2.7us table load.
  - x and data are converted to fp16 on-device (rel err ~2.6e-3 total)
    and transposed DRAM->SBUF by the DMA xbar transpose engine: zero PE
    transposes, zero PSUM->SBUF copies, FWL-accelerated weight loads,
    and the output comes out in natural row order (no reorder pass).
  - Two passes over the j-tiles: pass h computes x-half [512h, 512h+512)
    for all 64 j-tiles; one 512-wide PSUM accumulator bank per pass
    (reused). PSUM map: pm [128,1536] x2 (6 banks) | acc (1) | x-norm
    scratch (1).
  - Data staging (fp32 chunk DMA -> DVE fp16 convert -> DMA out ->
    DMA-transpose in + DVE norms + w) is interleaved just-in-time into
    pass 1.
"""
import math
from contextlib import ExitStack

import numpy as np

from concourse import bacc, mybir, tile
from concourse.bass_utils import run_bass_kernel_spmd
from concourse import masks

N, M, D = 8192, 8192, 128
NCORES = 8
NS = N // NCORES            # 1024 x-rows per core
P = 128                     # partitions
NT_X = NS // P              # 8 x tiles
NT_D = M // P               # 64 data tiles
NCHUNK = 8                  # data DMA chunks
TPC = NT_D // NCHUNK        # 8 tiles per chunk

C = 0.5 / math.sqrt(2.0 * math.pi)
TWO_C = 2.0 * C
LNM = math.log(float(M))

F32 = mybir.dt.float32
F32R = mybir.dt.float32r
F16 = mybir.dt.float16

_CACHED_NC = None

JT_PER_TILE = 3                             # 1536-wide psum tile = 3 j-tiles
NTILE = (NT_D + JT_PER_TILE - 1) // JT_PER_TILE   # 22 tiles (21x3 + 1x1)

# chunk k of data staged (converted/transposed) just before this pass-1 tile:
# chunks 0-3 are transposed by the PE (low latency, fills the ramp);
# chunks 4-7 round-trip through DRAM via the DMA xbar transpose engine.
_STAGE_AT = {}
for _k in range(NCHUNK):
    _STAGE_AT.setdefault(max(0, (8 * _k) // JT_PER_TILE - 3) if _k < 4
                         else _k - 1, []).append(_k)


def _build():
    nc = bacc.Bacc("TRN2", target_bir_lowering=False, debug=False)
    x_d = nc.dram_tensor("x", [NS, D], F32, kind="ExternalInput")
    d_d = nc.dram_tensor("data", [M, D], F32, kind="ExternalInput")
    o_d = nc.dram_tensor("out", [1, NS], F32, kind="ExternalOutput")

    # inputs load contiguously: row p*T+r lands at [p, r] (T rows/partition)
    x_re = x_d.ap().rearrange("(p r) d -> p r d", p=P)     # [128, 8, 128]
    d_re = d_d.ap().rearrange("(s p) d -> p s d", p=P)     # [128, 64, 128]

    with tile.TileContext(nc) as tc, ExitStack() as ctx:
        const_pool = ctx.enter_context(tc.tile_pool(name="const", bufs=1))
        dT_pool = ctx.enter_context(tc.tile_pool(name="dT", bufs=1))
        xbuf_pool = ctx.enter_context(tc.tile_pool(name="xbuf", bufs=1))
        drow_pool = ctx.enter_context(tc.tile_pool(name="drow", bufs=3))
        f16_pool = ctx.enter_context(tc.tile_pool(name="f16", bufs=2))
        sq_pool = ctx.enter_context(tc.tile_pool(name="sq", bufs=2))
        e_pool = ctx.enter_context(tc.tile_pool(name="e", bufs=3))
        out_pool = ctx.enter_context(tc.tile_pool(name="outp", bufs=1))
        dram_pool = ctx.enter_context(tc.tile_pool(name="dscr", bufs=1, space="DRAM"))
        ps_main = ctx.enter_context(tc.tile_pool(name="psm", bufs=2, space="PSUM"))
        ps_acc = ctx.enter_context(tc.tile_pool(name="psa", bufs=1, space="PSUM"))
        ps_x = ctx.enter_context(tc.tile_pool(name="psx", bufs=1, space="PSUM"))

        ones_f = const_pool.tile([P, 1], F32, tag="onesf")
        nc.gpsimd.memset(ones_f[:], 1.0)
        ones_r = const_pool.tile([P, 1], F32R, tag="ones")
        nc.vector.tensor_copy(ones_r[:], ones_f[:])
        nlm_bias = const_pool.tile([1, 1], F32, tag="nlm")
        nc.gpsimd.memset(nlm_bias[:], -LNM)
        dummy = const_pool.tile([1, 1], F32, tag="dummy")
        ident = const_pool.tile([P, P], F32, tag="ident")
        masks.make_identity(nc, ident[:])

        dataT = dT_pool.tile([P, M], F16, tag="dataT")           # 16KB/part
        xT = xbuf_pool.tile([P, NS], F16, tag="xT")
        xsqT = xbuf_pool.tile([P, NS], F32R, tag="xsqT")
        xrow = xbuf_pool.tile([P, NT_X, P], F32, tag="xrow")
        dnsq = const_pool.tile([P, NT_D], F32, tag="dnsq")
        wj = const_pool.tile([P, NT_D], F32R, tag="wj")
        exf = out_pool.tile([1, NS], F32, tag="exf")
        dens = out_pool.tile([1, NS], F32, tag="dens")
        scr_d = dram_pool.tile([M, D], F16, tag="scrd")
        scr_x = dram_pool.tile([NS, D], F16, tag="scrx")

        # preload the exp table set while the input DMAs run
        nc.scalar.activation(dummy[:], ones_f[0:1, 0:1],
                             mybir.ActivationFunctionType.Exp)

        # ---- all input DMAs issued up front (x first: it is tiny) ----
        nc.sync.dma_start(xrow[:], x_re)
        drows = []
        for ch in range(NCHUNK):
            drow = drow_pool.tile([P, TPC, P], F32, tag="drow")
            nc.sync.dma_start(drow[:], d_re[:, ch * TPC:(ch + 1) * TPC, :])
            drows.append(drow)

        # ---- x: fp16 convert -> DRAM -> xbar transpose -> xT [128, 1024]
        xf16 = f16_pool.tile([P, NT_X, P], F16, tag="xf16")
        nc.vector.tensor_copy(xf16[:], xrow[:])
        nc.sync.dma_start(scr_x[:].rearrange("(p r) d -> p r d", p=P), xf16[:])
        nc.sync.dma_start_transpose(xT[:], scr_x[:])

        def stage_chunk(ch):
            """Stage chunk ch into fp16 dataT: chunks 0-3 via PE transposes
            through the spare psum bank, 4-7 via a DRAM round trip and the
            DMA xbar transpose; plus fused squared norms + w."""
            drow = drows[ch]
            if ch < 4:
                for b in range(2):
                    trd = ps_x.tile([P, 512], F32, tag="px")
                    for k in range(4):
                        nc.tensor.transpose(trd[:, k * P:(k + 1) * P],
                                            drow[:, b * 4 + k, :], ident[:])
                    base = (ch * TPC + b * 4) * P
                    nc.vector.tensor_copy(dataT[:, base:base + 512], trd[:])
            else:
                df16 = f16_pool.tile([P, TPC, P], F16, tag="df16")
                nc.vector.tensor_copy(df16[:], drow[:])
                rsl = slice(ch * TPC * P, (ch + 1) * TPC * P)
                nc.gpsimd.dma_start(
                    scr_d[rsl, :].rearrange("(s p) d -> p s d", p=P), df16[:])
                nc.sync.dma_start_transpose(dataT[:, rsl], scr_d[rsl, :])
            sq = sq_pool.tile([P, TPC, P], F32, tag="sq")
            nc.vector.tensor_mul(sq[:], drow[:], drow[:])
            nc.vector.tensor_reduce(
                dnsq[:, ch * TPC:(ch + 1) * TPC], sq[:],
                axis=mybir.AxisListType.X, op=mybir.AluOpType.add)
            nc.scalar.activation(
                wj[:, ch * TPC:(ch + 1) * TPC],
                dnsq[:, ch * TPC:(ch + 1) * TPC],
                mybir.ActivationFunctionType.Exp, scale=-C)

        # ---- main: two passes over j-tiles, pass h = x-half h ----
        for h in range(2):
            xsl = xT[:, h * 512:(h + 1) * 512]
            acc = ps_acc.tile([1, 512], F32, tag="acc")
            first = True
            for t in range(NTILE):
                if h == 0:
                    for ch in _STAGE_AT.get(t, ()):
                        stage_chunk(ch)
                jts = list(range(t * JT_PER_TILE,
                                 min((t + 1) * JT_PER_TILE, NT_D)))
                fd = 512 * len(jts)
                pm = ps_main.tile([P, 1536], F32, tag="pm")
                e = e_pool.tile([P, 1536], F32R, tag="e")
                for cix, jt in enumerate(jts):
                    nc.tensor.matmul(pm[:, cix * 512:(cix + 1) * 512],
                                     dataT[:, jt * P:(jt + 1) * P], xsl,
                                     start=True, stop=True)
                nc.scalar.activation(e[:, 0:fd], pm[:, 0:fd],
                                     mybir.ActivationFunctionType.Exp,
                                     scale=TWO_C)
                for cix, jt in enumerate(jts):
                    nc.tensor.matmul(acc[:], wj[:, jt:jt + 1],
                                     e[:, cix * 512:(cix + 1) * 512],
                                     start=first, stop=(jt == NT_D - 1),
                                     skip_group_check=True)
                    first = False
            if h == 0:
                # x-norm factor exf = exp(-C||x_i||^2 - lnM), off the ramp's
                # critical path (only needed by the pass epilogues)
                nc.vector.tensor_mul(xsqT[:], xT[:], xT[:])
                for c2 in range(2):
                    pmx = ps_x.tile([P, 512], F32, tag="px")
                    sl = slice(c2 * 512, (c2 + 1) * 512)
                    nc.tensor.matmul(pmx[0:1, :], ones_r[:], xsqT[:, sl],
                                     start=True, stop=True)
                    nc.scalar.activation(exf[:, sl], pmx[0:1, :],
                                         mybir.ActivationFunctionType.Exp,
                                         bias=nlm_bias[:], scale=-C)
            hsl = slice(h * 512, (h + 1) * 512)
            nc.vector.tensor_mul(dens[:, hsl], acc[:], exf[:, hsl])

        nc.sync.dma_start(o_d.ap(), dens[:])

    nc.compile()
    return nc


def kernel(x, data):
    global _CACHED_NC
    x = np.ascontiguousarray(np.asarray(x, dtype=np.float32))
    data = np.ascontiguousarray(np.asarray(data, dtype=np.float32))
    assert x.shape == (N, D) and data.shape == (M, D)

    if _CACHED_NC is None:
        _CACHED_NC = _build()
    nc = _CACHED_NC

    in_maps = [
        {"x": x[c * NS:(c + 1) * NS], "data": data} for c in range(NCORES)
    ]
    res = run_bass_kernel_spmd(nc, in_maps, list(range(NCORES)))
    dens = np.concatenate(
        [np.asarray(res.results[c]["out"]).reshape(NS) for c in range(NCORES)]
    )
    return dens.reshape(N, 1).astype(np.float32)


if __name__ == "__main__":
    rng = np.random.default_rng(0)
    x = rng.standard_normal((N, D), dtype=np.float32)
    data = rng.standard_normal((M, D), dtype=np.float32)
    out = kernel(x, data)
    print("kernel out", out.shape, out[:4, 0])


# revision 3
# speedup vs baseline: 1.0024x; 1.0024x over previous
"""Trainium2 Bass kernel for differentiable KDE (Gaussian kernel density estimate).

Math (h = 1):
    density[i] = exp(-C||x_i||^2 - ln M) * sum_j w_j * exp(2C x_i.d_j),
                 w_j = exp(-C||d_j||^2),  C = 0.5/sqrt(2*pi)

Sharding: data-parallel over x rows (1024 per core), data replicated.

v5 design notes:
  - ACT (scalar engine) is the hard bottleneck: exp runs at 1
    elem/lane/cycle + ~350 cycles/instruction. The d-norm factor w_j is
    applied as the stationary weight of the reduction matvec (not an ACT
    bias), so activations are biasless and one ACTIVATE spans 3 j-tiles
    (FD=1536). The exp table set is preloaded by a dummy activation at
    t=0 so the first real exp doesn't pay the ~2.7us table load.
  - x and data are converted to fp16 on-device (rel err ~2.6e-3 total)
    and transposed DRAM->SBUF by the DMA xbar transpose engine: zero PE
    transposes, zero PSUM->SBUF copies, FWL-accelerated weight loads,
    and the output comes out in natural row order (no reorder pass).
  - Two passes over the j-tiles: pass h computes x-half [512h, 512h+512)
    for all 64 j-tiles; one 512-wide PSUM accumulator bank per pass
    (reused). PSUM map: pm [128,1536] x2 (6 banks) | acc (1) | x-norm
    scratch (1).
  - Data staging (fp32 chunk DMA -> DVE fp16 convert -> DMA out ->
    DMA-transpose in + DVE norms + w) is interleaved just-in-time into
    pass 1.
"""
import math
from contextlib import ExitStack

import numpy as np

from concourse import bacc, mybir, tile
from concourse.bass_utils import run_bass_kernel_spmd
from concourse import masks

N, M, D = 8192, 8192, 128
NCORES = 8
NS = N // NCORES            # 1024 x-rows per core
P = 128                     # partitions
NT_X = NS // P              # 8 x tiles
NT_D = M // P               # 64 data tiles
NCHUNK = 8                  # data DMA chunks
TPC = NT_D // NCHUNK        # 8 tiles per chunk

C = 0.5 / math.sqrt(2.0 * math.pi)
TWO_C = 2.0 * C
LNM = math.log(float(M))

F32 = mybir.dt.float32
F32R = mybir.dt.float32r
F16 = mybir.dt.float16

_CACHED_NC = None

JT_PER_TILE = 3                             # 1536-wide psum tile = 3 j-tiles
NTILE = (NT_D + JT_PER_TILE - 1) // JT_PER_TILE   # 22 tiles (21x3 + 1x1)

# chunk k of data staged (converted/transposed) just before this pass-1 tile:
# chunks 0-3 are transposed by the PE (low latency, fills the ramp);
# chunks 4-7 round-trip through DRAM via the DMA xbar transpose engine.
_STAGE_AT = {}
for _k in range(NCHUNK):
    _STAGE_AT.setdefault(max(0, (8 * _k) // JT_PER_TILE - 3) if _k < 5
                         else _k - 1, []).append(_k)


def _build():
    nc = bacc.Bacc("TRN2", target_bir_lowering=False, debug=False)
    x_d = nc.dram_tensor("x", [NS, D], F32, kind="ExternalInput")
    d_d = nc.dram_tensor("data", [M, D], F32, kind="ExternalInput")
    o_d = nc.dram_tensor("out", [1, NS], F32, kind="ExternalOutput")

    # inputs load contiguously: row p*T+r lands at [p, r] (T rows/partition)
    x_re = x_d.ap().rearrange("(p r) d -> p r d", p=P)     # [128, 8, 128]
    d_re = d_d.ap().rearrange("(s p) d -> p s d", p=P)     # [128, 64, 128]

    with tile.TileContext(nc) as tc, ExitStack() as ctx:
        const_pool = ctx.enter_context(tc.tile_pool(name="const", bufs=1))
        dT_pool = ctx.enter_context(tc.tile_pool(name="dT", bufs=1))
        xbuf_pool = ctx.enter_context(tc.tile_pool(name="xbuf", bufs=1))
        drow_pool = ctx.enter_context(tc.tile_pool(name="drow", bufs=4))
        f16_pool = ctx.enter_context(tc.tile_pool(name="f16", bufs=2))
        sq_pool = ctx.enter_context(tc.tile_pool(name="sq", bufs=2))
        e_pool = ctx.enter_context(tc.tile_pool(name="e", bufs=4))
        out_pool = ctx.enter_context(tc.tile_pool(name="outp", bufs=1))
        dram_pool = ctx.enter_context(tc.tile_pool(name="dscr", bufs=1, space="DRAM"))
        ps_main = ctx.enter_context(tc.tile_pool(name="psm", bufs=2, space="PSUM"))
        ps_acc = ctx.enter_context(tc.tile_pool(name="psa", bufs=1, space="PSUM"))
        ps_x = ctx.enter_context(tc.tile_pool(name="psx", bufs=1, space="PSUM"))

        ones_f = const_pool.tile([P, 1], F32, tag="onesf")
        nc.gpsimd.memset(ones_f[:], 1.0)
        ones_r = const_pool.tile([P, 1], F32R, tag="ones")
        nc.vector.tensor_copy(ones_r[:], ones_f[:])
        nlm_bias = const_pool.tile([1, 1], F32, tag="nlm")
        nc.gpsimd.memset(nlm_bias[:], -LNM)
        dummy = const_pool.tile([1, 1], F32, tag="dummy")
        ident = const_pool.tile([P, P], F32, tag="ident")
        masks.make_identity(nc, ident[:])

        dataT = dT_pool.tile([P, M], F16, tag="dataT")           # 16KB/part
        xT = xbuf_pool.tile([P, NS], F16, tag="xT")
        xsqT = xbuf_pool.tile([P, NS], F32R, tag="xsqT")
        xrow = xbuf_pool.tile([P, NT_X, P], F32, tag="xrow")
        dnsq = const_pool.tile([P, NT_D], F32, tag="dnsq")
        wj = const_pool.tile([P, NT_D], F32R, tag="wj")
        exf = out_pool.tile([1, NS], F32, tag="exf")
        dens = out_pool.tile([1, NS], F32, tag="dens")
        scr_d = dram_pool.tile([M, D], F16, tag="scrd")
        scr_x = dram_pool.tile([NS, D], F16, tag="scrx")

        # preload the exp table set while the input DMAs run
        nc.scalar.activation(dummy[:], ones_f[0:1, 0:1],
                             mybir.ActivationFunctionType.Exp)

        # ---- all input DMAs issued up front (x first: it is tiny) ----
        nc.sync.dma_start(xrow[:], x_re)
        drows = []
        for ch in range(NCHUNK):
            drow = drow_pool.tile([P, TPC, P], F32, tag="drow")
            nc.sync.dma_start(drow[:], d_re[:, ch * TPC:(ch + 1) * TPC, :])
            drows.append(drow)

        # ---- x: fp16 convert -> DRAM -> xbar transpose -> xT [128, 1024]
        xf16 = f16_pool.tile([P, NT_X, P], F16, tag="xf16")
        nc.vector.tensor_copy(xf16[:], xrow[:])
        nc.scalar.dma_start(scr_x[:].rearrange("(p r) d -> p r d", p=P), xf16[:])
        nc.scalar.dma_start_transpose(xT[:], scr_x[:])

        def stage_chunk(ch):
            """Stage chunk ch into fp16 dataT: chunks 0-3 via PE transposes
            through the spare psum bank, 4-7 via a DRAM round trip and the
            DMA xbar transpose; plus fused squared norms + w."""
            drow = drows[ch]
            if ch < 5:
                for b in range(2):
                    trd = ps_x.tile([P, 512], F32, tag="px")
                    for k in range(4):
                        nc.tensor.transpose(trd[:, k * P:(k + 1) * P],
                                            drow[:, b * 4 + k, :], ident[:])
                    base = (ch * TPC + b * 4) * P
                    nc.vector.tensor_copy(dataT[:, base:base + 512], trd[:])
            else:
                df16 = f16_pool.tile([P, TPC, P], F16, tag="df16")
                nc.vector.tensor_copy(df16[:], drow[:])
                rsl = slice(ch * TPC * P, (ch + 1) * TPC * P)
                nc.gpsimd.dma_start(
                    scr_d[rsl, :].rearrange("(s p) d -> p s d", p=P), df16[:])
                nc.sync.dma_start_transpose(dataT[:, rsl], scr_d[rsl, :])
            sq = sq_pool.tile([P, TPC, P], F32, tag="sq")
            nc.vector.tensor_mul(sq[:], drow[:], drow[:])
            nc.vector.tensor_reduce(
                dnsq[:, ch * TPC:(ch + 1) * TPC], sq[:],
                axis=mybir.AxisListType.X, op=mybir.AluOpType.add)
            nc.scalar.activation(
                wj[:, ch * TPC:(ch + 1) * TPC],
                dnsq[:, ch * TPC:(ch + 1) * TPC],
                mybir.ActivationFunctionType.Exp, scale=-C)

        # ---- main: two passes over j-tiles, pass h = x-half h ----
        for h in range(2):
            xsl = xT[:, h * 512:(h + 1) * 512]
            acc = ps_acc.tile([1, 512], F32, tag="acc")
            first = True
            for t in range(NTILE):
                if h == 0:
                    for ch in _STAGE_AT.get(t, ()):
                        stage_chunk(ch)
                jts = list(range(t * JT_PER_TILE,
                                 min((t + 1) * JT_PER_TILE, NT_D)))
                fd = 512 * len(jts)
                pm = ps_main.tile([P, 1536], F32, tag="pm")
                e = e_pool.tile([P, 1536], F32R, tag="e")
                for cix, jt in enumerate(jts):
                    nc.tensor.matmul(pm[:, cix * 512:(cix + 1) * 512],
                                     dataT[:, jt * P:(jt + 1) * P], xsl,
                                     start=True, stop=True)
                nc.scalar.activation(e[:, 0:fd], pm[:, 0:fd],
                                     mybir.ActivationFunctionType.Exp,
                                     scale=TWO_C)
                for cix, jt in enumerate(jts):
                    nc.tensor.matmul(acc[:], wj[:, jt:jt + 1],
                                     e[:, cix * 512:(cix + 1) * 512],
                                     start=first, stop=(jt == NT_D - 1),
                                     skip_group_check=True)
                    first = False
            if h == 0:
                # x-norm factor exf = exp(-C||x_i||^2 - lnM), off the ramp's
                # critical path (only needed by the pass epilogues)
                nc.vector.tensor_mul(xsqT[:], xT[:], xT[:])
                for c2 in range(2):
                    pmx = ps_x.tile([P, 512], F32, tag="px")
                    sl = slice(c2 * 512, (c2 + 1) * 512)
                    nc.tensor.matmul(pmx[0:1, :], ones_r[:], xsqT[:, sl],
                                     start=True, stop=True)
                    nc.scalar.activation(exf[:, sl], pmx[0:1, :],
                                         mybir.ActivationFunctionType.Exp,
                                         bias=nlm_bias[:], scale=-C)
            hsl = slice(h * 512, (h + 1) * 512)
            nc.vector.tensor_mul(dens[:, hsl], acc[:], exf[:, hsl])

        nc.sync.dma_start(o_d.ap(), dens[:])

    nc.compile()
    return nc


def kernel(x, data):
    global _CACHED_NC
    x = np.ascontiguousarray(np.asarray(x, dtype=np.float32))
    data = np.ascontiguousarray(np.asarray(data, dtype=np.float32))
    assert x.shape == (N, D) and data.shape == (M, D)

    if _CACHED_NC is None:
        _CACHED_NC = _build()
    nc = _CACHED_NC

    in_maps = [
        {"x": x[c * NS:(c + 1) * NS], "data": data} for c in range(NCORES)
    ]
    res = run_bass_kernel_spmd(nc, in_maps, list(range(NCORES)))
    dens = np.concatenate(
        [np.asarray(res.results[c]["out"]).reshape(NS) for c in range(NCORES)]
    )
    return dens.reshape(N, 1).astype(np.float32)


if __name__ == "__main__":
    rng = np.random.default_rng(0)
    x = rng.standard_normal((N, D), dtype=np.float32)
    data = rng.standard_normal((M, D), dtype=np.float32)
    out = kernel(x, data)
    print("kernel out", out.shape, out[:4, 0])


# revision 4
# speedup vs baseline: 1.0192x; 1.0168x over previous
"""Trainium2 Bass kernel for differentiable KDE (Gaussian kernel density estimate).

Math (h = 1):
    density[i] = exp(-C||x_i||^2 - ln M) * sum_j w_j * exp(2C x_i.d_j),
                 w_j = exp(-C||d_j||^2),  C = 0.5/sqrt(2*pi)

Sharding: data-parallel over x rows (1024 per core), data replicated.

v5 design notes:
  - ACT (scalar engine) is the hard bottleneck: exp runs at 1
    elem/lane/cycle + ~350 cycles/instruction. The d-norm factor w_j is
    applied as the stationary weight of the reduction matvec (not an ACT
    bias), so activations are biasless and one ACTIVATE spans 3 j-tiles
    (FD=1536). The exp table set is preloaded by a dummy activation at
    t=0 so the first real exp doesn't pay the ~2.7us table load.
  - x and data are converted to fp16 on-device (rel err ~2.6e-3 total)
    and transposed DRAM->SBUF by the DMA xbar transpose engine: zero PE
    transposes, zero PSUM->SBUF copies, FWL-accelerated weight loads,
    and the output comes out in natural row order (no reorder pass).
  - Two passes over the j-tiles: pass h computes x-half [512h, 512h+512)
    for all 64 j-tiles; one 512-wide PSUM accumulator bank per pass
    (reused). PSUM map: pm [128,1536] x2 (6 banks) | acc (1) | x-norm
    scratch (1).
  - Data staging (fp32 chunk DMA -> DVE fp16 convert -> DMA out ->
    DMA-transpose in + DVE norms + w) is interleaved just-in-time into
    pass 1.
"""
import math
from contextlib import ExitStack

import numpy as np

from concourse import bacc, mybir, tile
from concourse.bass_utils import run_bass_kernel_spmd
from concourse import masks

N, M, D = 8192, 8192, 128
NCORES = 8
NS = N // NCORES            # 1024 x-rows per core
P = 128                     # partitions
NT_X = NS // P              # 8 x tiles
NT_D = M // P               # 64 data tiles
NCHUNK = 8                  # data DMA chunks
TPC = NT_D // NCHUNK        # 8 tiles per chunk

C = 0.5 / math.sqrt(2.0 * math.pi)
TWO_C = 2.0 * C
LNM = math.log(float(M))

F32 = mybir.dt.float32
F32R = mybir.dt.float32r
F16 = mybir.dt.float16

_CACHED_NC = None

JT_PER_TILE = 3                             # 1536-wide psum tile = 3 j-tiles
NTILE = (NT_D + JT_PER_TILE - 1) // JT_PER_TILE   # 22 tiles (21x3 + 1x1)

# chunk k of data staged (converted/transposed) just before this pass-1 tile:
# chunks 0-3 are transposed by the PE (low latency, fills the ramp);
# chunks 4-7 round-trip through DRAM via the DMA xbar transpose engine.
_STAGE_AT = {}
for _k in range(NCHUNK):
    _STAGE_AT.setdefault(max(0, (8 * _k) // JT_PER_TILE - 3) if _k < 5
                         else _k - 1, []).append(_k)


def _build():
    nc = bacc.Bacc("TRN2", target_bir_lowering=False, debug=False)
    x_d = nc.dram_tensor("x", [NS, D], F32, kind="ExternalInput")
    d_d = nc.dram_tensor("data", [M, D], F32, kind="ExternalInput")
    o_d = nc.dram_tensor("out", [1, NS], F32, kind="ExternalOutput")

    # inputs load contiguously: row p*T+r lands at [p, r] (T rows/partition)
    x_re = x_d.ap().rearrange("(p r) d -> p r d", p=P)     # [128, 8, 128]
    d_re = d_d.ap().rearrange("(s p) d -> p s d", p=P)     # [128, 64, 128]

    with tile.TileContext(nc) as tc, ExitStack() as ctx:
        const_pool = ctx.enter_context(tc.tile_pool(name="const", bufs=1))
        dT_pool = ctx.enter_context(tc.tile_pool(name="dT", bufs=1))
        xbuf_pool = ctx.enter_context(tc.tile_pool(name="xbuf", bufs=1))
        drow_pool = ctx.enter_context(tc.tile_pool(name="drow", bufs=4))
        f16_pool = ctx.enter_context(tc.tile_pool(name="f16", bufs=2))
        sq_pool = ctx.enter_context(tc.tile_pool(name="sq", bufs=2))
        e_pool = ctx.enter_context(tc.tile_pool(name="e", bufs=4))
        out_pool = ctx.enter_context(tc.tile_pool(name="outp", bufs=1))
        dram_pool = ctx.enter_context(tc.tile_pool(name="dscr", bufs=1, space="DRAM"))
        ps_main = ctx.enter_context(tc.tile_pool(name="psm", bufs=2, space="PSUM"))
        ps_acc = ctx.enter_context(tc.tile_pool(name="psa", bufs=1, space="PSUM"))
        ps_x = ctx.enter_context(tc.tile_pool(name="psx", bufs=1, space="PSUM"))

        ones_f = const_pool.tile([P, 1], F32, tag="onesf")
        nc.gpsimd.memset(ones_f[:], 1.0)
        ones_r = const_pool.tile([P, 1], F32R, tag="ones")
        nc.vector.tensor_copy(ones_r[:], ones_f[:])
        nlm_bias = const_pool.tile([1, 1], F32, tag="nlm")
        nc.gpsimd.memset(nlm_bias[:], -LNM)
        dummy = const_pool.tile([1, 1], F32, tag="dummy")
        ident = const_pool.tile([P, P], F32, tag="ident")
        masks.make_identity(nc, ident[:])

        dataT = dT_pool.tile([P, M], F16, tag="dataT")           # 16KB/part
        xT = xbuf_pool.tile([P, NS], F16, tag="xT")
        xsqT = xbuf_pool.tile([P, NS], F32R, tag="xsqT")
        xrow = xbuf_pool.tile([P, NT_X, P], F32, tag="xrow")
        dnsq = const_pool.tile([P, NT_D], F32, tag="dnsq")
        wj = const_pool.tile([P, NT_D], F32R, tag="wj")
        exf = out_pool.tile([1, NS], F32, tag="exf")
        dens = out_pool.tile([1, NS], F32, tag="dens")
        scr_d = dram_pool.tile([M, D], F16, tag="scrd")

        # preload the exp table set while the input DMAs run
        nc.scalar.activation(dummy[:], ones_f[0:1, 0:1],
                             mybir.ActivationFunctionType.Exp)

        # ---- all input DMAs issued up front (x first: it is tiny) ----
        nc.sync.dma_start(xrow[:], x_re)
        drows = []
        for ch in range(NCHUNK):
            drow = drow_pool.tile([P, TPC, P], F32, tag="drow")
            nc.sync.dma_start(drow[:], d_re[:, ch * TPC:(ch + 1) * TPC, :])
            drows.append(drow)

        # ---- x: PE transposes through the spare psum bank (the DMA round
        # trip costs ~16us of latency; the induced column permutation is
        # undone by one reorder copy of the [1, 1024] result at the end)
        for b in range(2):
            trx = ps_x.tile([P, 512], F32, tag="px")
            for k in range(4):
                t = b * 4 + k
                nc.tensor.transpose(trx[:, k * P:(k + 1) * P], xrow[:, t, :],
                                    ident[:])
            nc.vector.tensor_copy(xT[:, b * 512:(b + 1) * 512], trx[:])

        def stage_chunk(ch):
            """Stage chunk ch into fp16 dataT: chunks 0-3 via PE transposes
            through the spare psum bank, 4-7 via a DRAM round trip and the
            DMA xbar transpose; plus fused squared norms + w."""
            drow = drows[ch]
            if ch < 5:
                for b in range(2):
                    trd = ps_x.tile([P, 512], F32, tag="px")
                    for k in range(4):
                        nc.tensor.transpose(trd[:, k * P:(k + 1) * P],
                                            drow[:, b * 4 + k, :], ident[:])
                    base = (ch * TPC + b * 4) * P
                    nc.vector.tensor_copy(dataT[:, base:base + 512], trd[:])
            else:
                df16 = f16_pool.tile([P, TPC, P], F16, tag="df16")
                nc.vector.tensor_copy(df16[:], drow[:])
                rsl = slice(ch * TPC * P, (ch + 1) * TPC * P)
                nc.gpsimd.dma_start(
                    scr_d[rsl, :].rearrange("(s p) d -> p s d", p=P), df16[:])
                nc.sync.dma_start_transpose(dataT[:, rsl], scr_d[rsl, :])
            sq = sq_pool.tile([P, TPC, P], F32, tag="sq")
            nc.vector.tensor_mul(sq[:], drow[:], drow[:])
            nc.vector.tensor_reduce(
                dnsq[:, ch * TPC:(ch + 1) * TPC], sq[:],
                axis=mybir.AxisListType.X, op=mybir.AluOpType.add)
            nc.scalar.activation(
                wj[:, ch * TPC:(ch + 1) * TPC],
                dnsq[:, ch * TPC:(ch + 1) * TPC],
                mybir.ActivationFunctionType.Exp, scale=-C)

        # ---- main: two passes over j-tiles, pass h = x-half h ----
        for h in range(2):
            xsl = xT[:, h * 512:(h + 1) * 512]
            acc = ps_acc.tile([1, 512], F32, tag="acc")
            first = True
            for t in range(NTILE):
                if h == 0:
                    for ch in _STAGE_AT.get(t, ()):
                        stage_chunk(ch)
                jts = list(range(t * JT_PER_TILE,
                                 min((t + 1) * JT_PER_TILE, NT_D)))
                fd = 512 * len(jts)
                pm = ps_main.tile([P, 1536], F32, tag="pm")
                e = e_pool.tile([P, 1536], F32R, tag="e")
                for cix, jt in enumerate(jts):
                    nc.tensor.matmul(pm[:, cix * 512:(cix + 1) * 512],
                                     dataT[:, jt * P:(jt + 1) * P], xsl,
                                     start=True, stop=True)
                nc.scalar.activation(e[:, 0:fd], pm[:, 0:fd],
                                     mybir.ActivationFunctionType.Exp,
                                     scale=TWO_C)
                for cix, jt in enumerate(jts):
                    nc.tensor.matmul(acc[:], wj[:, jt:jt + 1],
                                     e[:, cix * 512:(cix + 1) * 512],
                                     start=first, stop=(jt == NT_D - 1),
                                     skip_group_check=True)
                    first = False
            if h == 0:
                # x-norm factor exf = exp(-C||x_i||^2 - lnM), off the ramp's
                # critical path (only needed by the pass epilogues)
                nc.vector.tensor_mul(xsqT[:], xT[:], xT[:])
                for c2 in range(2):
                    pmx = ps_x.tile([P, 512], F32, tag="px")
                    sl = slice(c2 * 512, (c2 + 1) * 512)
                    nc.tensor.matmul(pmx[0:1, :], ones_r[:], xsqT[:, sl],
                                     start=True, stop=True)
                    nc.scalar.activation(exf[:, sl], pmx[0:1, :],
                                         mybir.ActivationFunctionType.Exp,
                                         bias=nlm_bias[:], scale=-C)
            hsl = slice(h * 512, (h + 1) * 512)
            nc.vector.tensor_mul(dens[:, hsl], acc[:], exf[:, hsl])

        # undo the x row permutation: dens index r*128+p -> row 8p+r
        dens_o = out_pool.tile([1, NS], F32, tag="dens_o")
        nc.vector.tensor_copy(
            dens_o[:], dens[:].rearrange("o (r p) -> o p r", p=P))
        nc.sync.dma_start(o_d.ap(), dens_o[:])

    nc.compile()
    return nc


def kernel(x, data):
    global _CACHED_NC
    x = np.ascontiguousarray(np.asarray(x, dtype=np.float32))
    data = np.ascontiguousarray(np.asarray(data, dtype=np.float32))
    assert x.shape == (N, D) and data.shape == (M, D)

    if _CACHED_NC is None:
        _CACHED_NC = _build()
    nc = _CACHED_NC

    in_maps = [
        {"x": x[c * NS:(c + 1) * NS], "data": data} for c in range(NCORES)
    ]
    res = run_bass_kernel_spmd(nc, in_maps, list(range(NCORES)))
    dens = np.concatenate(
        [np.asarray(res.results[c]["out"]).reshape(NS) for c in range(NCORES)]
    )
    return dens.reshape(N, 1).astype(np.float32)


if __name__ == "__main__":
    rng = np.random.default_rng(0)
    x = rng.standard_normal((N, D), dtype=np.float32)
    data = rng.standard_normal((M, D), dtype=np.float32)
    out = kernel(x, data)
    print("kernel out", out.shape, out[:4, 0])


# revision 6
# speedup vs baseline: 1.0408x; 1.0212x over previous
"""Trainium2 Bass kernel for differentiable KDE (Gaussian kernel density estimate).

Math (h = 1):
    density[i] = exp(-C||x_i||^2 - ln M) * sum_j w_j * exp(2C x_i.d_j),
                 w_j = exp(-C||d_j||^2),  C = 0.5/sqrt(2*pi)

Sharding: data-parallel over x rows (1024 per core), data replicated.

v5 design notes:
  - ACT (scalar engine) is the hard bottleneck: exp runs at 1
    elem/lane/cycle + ~350 cycles/instruction. The d-norm factor w_j is
    applied as the stationary weight of the reduction matvec (not an ACT
    bias), so activations are biasless and one ACTIVATE spans 3 j-tiles
    (FD=1536). The exp table set is preloaded by a dummy activation at
    t=0 so the first real exp doesn't pay the ~2.7us table load.
  - x and data are converted to fp16 on-device (rel err ~2.6e-3 total)
    and transposed DRAM->SBUF by the DMA xbar transpose engine: zero PE
    transposes, zero PSUM->SBUF copies, FWL-accelerated weight loads,
    and the output comes out in natural row order (no reorder pass).
  - Two passes over the j-tiles: pass h computes x-half [512h, 512h+512)
    for all 64 j-tiles; one 512-wide PSUM accumulator bank per pass
    (reused). PSUM map: pm [128,1536] x2 (6 banks) | acc (1) | x-norm
    scratch (1).
  - Data staging (fp32 chunk DMA -> DVE fp16 convert -> DMA out ->
    DMA-transpose in + DVE norms + w) is interleaved just-in-time into
    pass 1.
"""
import math
from contextlib import ExitStack

import numpy as np

from concourse import bacc, mybir, tile
from concourse.bass_utils import run_bass_kernel_spmd
from concourse import masks

N, M, D = 8192, 8192, 128
NCORES = 8
NS = N // NCORES            # 1024 x-rows per core
P = 128                     # partitions
NT_X = NS // P              # 8 x tiles
NT_D = M // P               # 64 data tiles
NCHUNK = 8                  # data DMA chunks
TPC = NT_D // NCHUNK        # 8 tiles per chunk

C = 0.5 / math.sqrt(2.0 * math.pi)
TWO_C = 2.0 * C
LNM = math.log(float(M))

F32 = mybir.dt.float32
F32R = mybir.dt.float32r
F16 = mybir.dt.float16

_CACHED_NC = None

JT_PER_TILE = 3                             # 1536-wide psum tile = 3 j-tiles
NTILE = (NT_D + JT_PER_TILE - 1) // JT_PER_TILE   # 22 tiles (21x3 + 1x1)

# chunk k of data staged (converted/transposed) just before this pass-1 tile:
# chunks 0-3 are transposed by the PE (low latency, fills the ramp);
# chunks 4-7 round-trip through DRAM via the DMA xbar transpose engine.
_STAGE_AT = {}
for _k in range(NCHUNK):
    _STAGE_AT.setdefault(max(0, (8 * _k) // JT_PER_TILE - 3) if _k < 5
                         else _k - 1, []).append(_k)


def _build():
    nc = bacc.Bacc("TRN2", target_bir_lowering=False, debug=False)
    x_d = nc.dram_tensor("x", [NS, D], F32, kind="ExternalInput")
    d_d = nc.dram_tensor("data", [M, D], F32, kind="ExternalInput")
    o_d = nc.dram_tensor("out", [1, NS], F32, kind="ExternalOutput")

    # inputs load contiguously: row p*T+r lands at [p, r] (T rows/partition)
    x_re = x_d.ap().rearrange("(p r) d -> p r d", p=P)     # [128, 8, 128]
    d_re = d_d.ap().rearrange("(s p) d -> p s d", p=P)     # [128, 64, 128]

    with tile.TileContext(nc) as tc, ExitStack() as ctx:
        const_pool = ctx.enter_context(tc.tile_pool(name="const", bufs=1))
        dT_pool = ctx.enter_context(tc.tile_pool(name="dT", bufs=1))
        xbuf_pool = ctx.enter_context(tc.tile_pool(name="xbuf", bufs=1))
        drow_pool = ctx.enter_context(tc.tile_pool(name="drow", bufs=4))
        f16_pool = ctx.enter_context(tc.tile_pool(name="f16", bufs=2))
        sq_pool = ctx.enter_context(tc.tile_pool(name="sq", bufs=2))
        e_pool = ctx.enter_context(tc.tile_pool(name="e", bufs=4))
        out_pool = ctx.enter_context(tc.tile_pool(name="outp", bufs=1))
        dram_pool = ctx.enter_context(tc.tile_pool(name="dscr", bufs=1, space="DRAM"))
        ps_main = ctx.enter_context(tc.tile_pool(name="psm", bufs=2, space="PSUM"))
        ps_acc = ctx.enter_context(tc.tile_pool(name="psa", bufs=1, space="PSUM"))
        ps_x = ctx.enter_context(tc.tile_pool(name="psx", bufs=1, space="PSUM"))

        dataT = dT_pool.tile([P, M], F16, tag="dataT")           # 16KB/part
        xT = xbuf_pool.tile([P, NS], F16, tag="xT")
        xsqT = xbuf_pool.tile([P, NS], F32R, tag="xsqT")
        xrow = xbuf_pool.tile([P, NT_X, P], F32, tag="xrow")
        dnsq = const_pool.tile([P, NT_D], F32, tag="dnsq")
        wj = const_pool.tile([P, NT_D], F32R, tag="wj")
        exf = out_pool.tile([1, NS], F32, tag="exf")
        dens = out_pool.tile([1, NS], F32, tag="dens")
        dens_o = out_pool.tile([1, NS], F32, tag="dens_o")
        scr_d = dram_pool.tile([M, D], F16, tag="scrd")

        # ---- all input DMAs issued first (before the const-setup work
        # that otherwise gates the sync queue); x first: it is tiny ----
        nc.sync.dma_start(xrow[:], x_re)
        drows = []
        for ch in range(NCHUNK):
            drow = drow_pool.tile([P, TPC, P], F32, tag="drow")
            nc.sync.dma_start(drow[:], d_re[:, ch * TPC:(ch + 1) * TPC, :])
            drows.append(drow)

        ones_f = const_pool.tile([P, 1], F32, tag="onesf")
        nc.gpsimd.memset(ones_f[:], 1.0)
        ones_r = const_pool.tile([P, 1], F32R, tag="ones")
        nc.vector.tensor_copy(ones_r[:], ones_f[:])
        nlm_bias = const_pool.tile([1, 1], F32, tag="nlm")
        nc.gpsimd.memset(nlm_bias[:], -LNM)
        dummy = const_pool.tile([1, 1], F32, tag="dummy")
        ident = const_pool.tile([P, P], F32, tag="ident")
        masks.make_identity(nc, ident[:])

        # preload the exp table set while the input DMAs run
        nc.scalar.activation(dummy[:], ones_f[0:1, 0:1],
                             mybir.ActivationFunctionType.Exp)



        # ---- x: PE transposes through the spare psum bank (the DMA round
        # trip costs ~16us of latency; the induced column permutation is
        # undone by one reorder copy of the [1, 1024] result at the end)
        for b in range(2):
            trx = ps_x.tile([P, 512], F32, tag="px")
            for k in range(4):
                t = b * 4 + k
                nc.tensor.transpose(trx[:, k * P:(k + 1) * P], xrow[:, t, :],
                                    ident[:])
            nc.vector.tensor_copy(xT[:, b * 512:(b + 1) * 512], trx[:])

        def stage_chunk(ch):
            """Stage chunk ch into fp16 dataT: chunks 0-3 via PE transposes
            through the spare psum bank, 4-7 via a DRAM round trip and the
            DMA xbar transpose; plus fused squared norms + w."""
            drow = drows[ch]
            if ch < 5:
                for b in range(2):
                    trd = ps_x.tile([P, 512], F32, tag="px")
                    for k in range(4):
                        nc.tensor.transpose(trd[:, k * P:(k + 1) * P],
                                            drow[:, b * 4 + k, :], ident[:])
                    base = (ch * TPC + b * 4) * P
                    nc.vector.tensor_copy(dataT[:, base:base + 512], trd[:])
            else:
                df16 = f16_pool.tile([P, TPC, P], F16, tag="df16")
                nc.vector.tensor_copy(df16[:], drow[:])
                rsl = slice(ch * TPC * P, (ch + 1) * TPC * P)
                nc.gpsimd.dma_start(
                    scr_d[rsl, :].rearrange("(s p) d -> p s d", p=P), df16[:])
                nc.sync.dma_start_transpose(dataT[:, rsl], scr_d[rsl, :])
            sq = sq_pool.tile([P, TPC, P], F32, tag="sq")
            nc.vector.tensor_mul(sq[:], drow[:], drow[:])
            nc.vector.tensor_reduce(
                dnsq[:, ch * TPC:(ch + 1) * TPC], sq[:],
                axis=mybir.AxisListType.X, op=mybir.AluOpType.add)
            nc.scalar.activation(
                wj[:, ch * TPC:(ch + 1) * TPC],
                dnsq[:, ch * TPC:(ch + 1) * TPC],
                mybir.ActivationFunctionType.Exp, scale=-C)

        # ---- main: two passes over j-tiles, pass h = x-half h ----
        for h in range(2):
            xsl = xT[:, h * 512:(h + 1) * 512]
            acc = ps_acc.tile([1, 512], F32, tag="acc")
            first = True
            for t in range(NTILE):
                if h == 0:
                    for ch in _STAGE_AT.get(t, ()):
                        stage_chunk(ch)
                jts = list(range(t * JT_PER_TILE,
                                 min((t + 1) * JT_PER_TILE, NT_D)))
                fd = 512 * len(jts)
                pm = ps_main.tile([P, 1536], F32, tag="pm")
                e = e_pool.tile([P, 1536], F32R, tag="e")
                for cix, jt in enumerate(jts):
                    nc.tensor.matmul(pm[:, cix * 512:(cix + 1) * 512],
                                     dataT[:, jt * P:(jt + 1) * P], xsl,
                                     start=True, stop=True)
                nc.scalar.activation(e[:, 0:fd], pm[:, 0:fd],
                                     mybir.ActivationFunctionType.Exp,
                                     scale=TWO_C)
                for cix, jt in enumerate(jts):
                    nc.tensor.matmul(acc[:], wj[:, jt:jt + 1],
                                     e[:, cix * 512:(cix + 1) * 512],
                                     start=first, stop=(jt == NT_D - 1),
                                     skip_group_check=True)
                    first = False
            if h == 0:
                # x-norm factor exf = exp(-C||x_i||^2 - lnM), off the ramp's
                # critical path (only needed by the pass epilogues)
                nc.vector.tensor_mul(xsqT[:], xT[:], xT[:])
                for c2 in range(2):
                    pmx = ps_x.tile([P, 512], F32, tag="px")
                    sl = slice(c2 * 512, (c2 + 1) * 512)
                    nc.tensor.matmul(pmx[0:1, :], ones_r[:], xsqT[:, sl],
                                     start=True, stop=True)
                    nc.scalar.activation(exf[:, sl], pmx[0:1, :],
                                         mybir.ActivationFunctionType.Exp,
                                         bias=nlm_bias[:], scale=-C)
            hsl = slice(h * 512, (h + 1) * 512)
            nc.vector.tensor_mul(dens[:, hsl], acc[:], exf[:, hsl])
            # undo the x row permutation for this half: col r*128+p -> row
            # 8p+r; half h covers r in [4h, 4h+4)
            nc.vector.tensor_copy(
                dens_o[:].rearrange("o (p r) -> o p r", r=8)[:, :, 4 * h:4 * h + 4],
                dens[:, hsl].rearrange("o (r p) -> o p r", p=P))

        nc.sync.dma_start(o_d.ap(), dens_o[:])

    nc.compile()
    return nc


def kernel(x, data):
    global _CACHED_NC
    x = np.ascontiguousarray(np.asarray(x, dtype=np.float32))
    data = np.ascontiguousarray(np.asarray(data, dtype=np.float32))
    assert x.shape == (N, D) and data.shape == (M, D)

    if _CACHED_NC is None:
        _CACHED_NC = _build()
    nc = _CACHED_NC

    in_maps = [
        {"x": x[c * NS:(c + 1) * NS], "data": data} for c in range(NCORES)
    ]
    res = run_bass_kernel_spmd(nc, in_maps, list(range(NCORES)))
    dens = np.concatenate(
        [np.asarray(res.results[c]["out"]).reshape(NS) for c in range(NCORES)]
    )
    return dens.reshape(N, 1).astype(np.float32)


if __name__ == "__main__":
    rng = np.random.default_rng(0)
    x = rng.standard_normal((N, D), dtype=np.float32)
    data = rng.standard_normal((M, D), dtype=np.float32)
    out = kernel(x, data)
    print("kernel out", out.shape, out[:4, 0])


# revision 7
# speedup vs baseline: 1.0638x; 1.0221x over previous
"""Trainium2 Bass kernel for differentiable KDE (Gaussian kernel density estimate).

Math (h = 1):
    density[i] = exp(-C||x_i||^2 - ln M) * sum_j w_j * exp(2C x_i.d_j),
                 w_j = exp(-C||d_j||^2),  C = 0.5/sqrt(2*pi)

Sharding: data-parallel over x rows (1024 per core), data replicated.

v5 design notes:
  - ACT (scalar engine) is the hard bottleneck: exp runs at 1
    elem/lane/cycle + ~350 cycles/instruction. The d-norm factor w_j is
    applied as the stationary weight of the reduction matvec (not an ACT
    bias), so activations are biasless and one ACTIVATE spans 3 j-tiles
    (FD=1536). The exp table set is preloaded by a dummy activation at
    t=0 so the first real exp doesn't pay the ~2.7us table load.
  - x and data are converted to fp16 on-device (rel err ~2.6e-3 total)
    and transposed DRAM->SBUF by the DMA xbar transpose engine: zero PE
    transposes, zero PSUM->SBUF copies, FWL-accelerated weight loads,
    and the output comes out in natural row order (no reorder pass).
  - Two passes over the j-tiles: pass h computes x-half [512h, 512h+512)
    for all 64 j-tiles; one 512-wide PSUM accumulator bank per pass
    (reused). PSUM map: pm [128,1536] x2 (6 banks) | acc (1) | x-norm
    scratch (1).
  - Data staging (fp32 chunk DMA -> DVE fp16 convert -> DMA out ->
    DMA-transpose in + DVE norms + w) is interleaved just-in-time into
    pass 1.
"""
import math
from contextlib import ExitStack

import numpy as np

from concourse import bacc, mybir, tile
from concourse.bass_utils import run_bass_kernel_spmd
from concourse import masks

N, M, D = 8192, 8192, 128
NCORES = 8
NS = N // NCORES            # 1024 x-rows per core
P = 128                     # partitions
NT_X = NS // P              # 8 x tiles
NT_D = M // P               # 64 data tiles
NCHUNK = 8                  # data DMA chunks
TPC = NT_D // NCHUNK        # 8 tiles per chunk

C = 0.5 / math.sqrt(2.0 * math.pi)
TWO_C = 2.0 * C
LNM = math.log(float(M))

F32 = mybir.dt.float32
F32R = mybir.dt.float32r
F16 = mybir.dt.float16

_CACHED_NC = None

JT_PER_TILE = 3                             # 1536-wide psum tile = 3 j-tiles
NTILE = (NT_D + JT_PER_TILE - 1) // JT_PER_TILE   # 22 tiles (21x3 + 1x1)

# chunk k of data staged (converted/transposed) just before this pass-1 tile:
# chunks 0-3 are transposed by the PE (low latency, fills the ramp);
# chunks 4-7 round-trip through DRAM via the DMA xbar transpose engine.
_STAGE_AT = {}
for _k in range(NCHUNK):
    _STAGE_AT.setdefault(max(0, (8 * _k) // JT_PER_TILE - 3) if _k < 5
                         else _k - 1, []).append(_k)


def _build():
    nc = bacc.Bacc("TRN2", target_bir_lowering=False, debug=False)
    x_d = nc.dram_tensor("x", [NS, D], F32, kind="ExternalInput")
    d_d = nc.dram_tensor("data", [M, D], F32, kind="ExternalInput")
    o_d = nc.dram_tensor("out", [1, NS], F32, kind="ExternalOutput")

    # inputs load contiguously: row p*T+r lands at [p, r] (T rows/partition)
    x_re = x_d.ap().rearrange("(p r) d -> p r d", p=P)     # [128, 8, 128]
    d_re = d_d.ap().rearrange("(s p) d -> p s d", p=P)     # [128, 64, 128]

    with tile.TileContext(nc) as tc, ExitStack() as ctx:
        const_pool = ctx.enter_context(tc.tile_pool(name="const", bufs=1))
        dT_pool = ctx.enter_context(tc.tile_pool(name="dT", bufs=1))
        xbuf_pool = ctx.enter_context(tc.tile_pool(name="xbuf", bufs=1))
        drow_pool = ctx.enter_context(tc.tile_pool(name="drow", bufs=4))
        f16_pool = ctx.enter_context(tc.tile_pool(name="f16", bufs=2))
        sq_pool = ctx.enter_context(tc.tile_pool(name="sq", bufs=2))
        e_pool = ctx.enter_context(tc.tile_pool(name="e", bufs=4))
        out_pool = ctx.enter_context(tc.tile_pool(name="outp", bufs=1))
        dram_pool = ctx.enter_context(tc.tile_pool(name="dscr", bufs=1, space="DRAM"))
        ps_main = ctx.enter_context(tc.tile_pool(name="psm", bufs=2, space="PSUM"))
        ps_acc = ctx.enter_context(tc.tile_pool(name="psa", bufs=1, space="PSUM"))
        ps_x = ctx.enter_context(tc.tile_pool(name="psx", bufs=1, space="PSUM"))

        dataT = dT_pool.tile([P, M], F16, tag="dataT")           # 16KB/part
        xT = xbuf_pool.tile([P, NS], F16, tag="xT")
        xsqT = xbuf_pool.tile([P, NS], F32R, tag="xsqT")
        xrow = xbuf_pool.tile([P, NT_X, P], F32, tag="xrow")
        dnsq = const_pool.tile([P, NT_D], F32, tag="dnsq")
        wj = const_pool.tile([P, NT_D], F32R, tag="wj")
        exf = out_pool.tile([1, NS], F32, tag="exf")
        dens = out_pool.tile([1, NS], F32, tag="dens")
        dens_o = out_pool.tile([1, NS], F32, tag="dens_o")
        scr_d = dram_pool.tile([M, D], F16, tag="scrd")

        # ---- all input DMAs issued first (before the const-setup work
        # that otherwise gates the sync queue); x first: it is tiny ----
        nc.sync.dma_start(xrow[:], x_re)
        drows = []
        for ch in range(NCHUNK):
            drow = drow_pool.tile([P, TPC, P], F32, tag="drow")
            nc.sync.dma_start(drow[:], d_re[:, ch * TPC:(ch + 1) * TPC, :])
            drows.append(drow)

        ones_f = const_pool.tile([P, 1], F32, tag="onesf")
        nc.gpsimd.memset(ones_f[:], 1.0)
        ones_r = const_pool.tile([P, 1], F32R, tag="ones")
        nc.vector.tensor_copy(ones_r[:], ones_f[:])
        nlm_bias = const_pool.tile([1, 1], F32, tag="nlm")
        nc.gpsimd.memset(nlm_bias[:], -LNM)
        dummy = const_pool.tile([1, 1], F32, tag="dummy")
        ident = const_pool.tile([P, P], F32, tag="ident")
        masks.make_identity(nc, ident[:])

        # preload the exp table set while the input DMAs run
        nc.scalar.activation(dummy[:], ones_f[0:1, 0:1],
                             mybir.ActivationFunctionType.Exp)



        # ---- x: PE transposes through the spare psum bank (the DMA round
        # trip costs ~16us of latency; the induced column permutation is
        # undone by one reorder copy of the [1, 1024] result at the end)
        for b in range(2):
            trx = ps_x.tile([P, 512], F32, tag="px")
            for k in range(4):
                t = b * 4 + k
                nc.tensor.transpose(trx[:, k * P:(k + 1) * P], xrow[:, t, :],
                                    ident[:])
            nc.vector.tensor_copy(xT[:, b * 512:(b + 1) * 512], trx[:])

        def stage_chunk(ch):
            """Stage chunk ch into fp16 dataT: chunks 0-3 via PE transposes
            through the spare psum bank, 4-7 via a DRAM round trip and the
            DMA xbar transpose; plus fused squared norms + w."""
            drow = drows[ch]
            if ch < 5:
                for b in range(2):
                    trd = ps_x.tile([P, 512], F32, tag="px")
                    for k in range(4):
                        nc.tensor.transpose(trd[:, k * P:(k + 1) * P],
                                            drow[:, b * 4 + k, :], ident[:])
                    base = (ch * TPC + b * 4) * P
                    nc.vector.tensor_copy(dataT[:, base:base + 512], trd[:])
            else:
                df16 = f16_pool.tile([P, TPC, P], F16, tag="df16")
                nc.vector.tensor_copy(df16[:], drow[:])
                rsl = slice(ch * TPC * P, (ch + 1) * TPC * P)
                nc.gpsimd.dma_start(
                    scr_d[rsl, :].rearrange("(s p) d -> p s d", p=P), df16[:])
                nc.sync.dma_start_transpose(dataT[:, rsl], scr_d[rsl, :])
            sq = sq_pool.tile([P, TPC, P], F32, tag="sq")
            nc.vector.tensor_mul(sq[:], drow[:], drow[:])
            nc.vector.tensor_reduce(
                dnsq[:, ch * TPC:(ch + 1) * TPC], sq[:],
                axis=mybir.AxisListType.X, op=mybir.AluOpType.add)
            nc.scalar.activation(
                wj[:, ch * TPC:(ch + 1) * TPC],
                dnsq[:, ch * TPC:(ch + 1) * TPC],
                mybir.ActivationFunctionType.Exp, scale=-C)

        # ---- main: two passes over j-tiles, pass h = x-half h ----
        for h in range(2):
            xsl = xT[:, h * 512:(h + 1) * 512]
            acc = ps_acc.tile([1, 512], F32, tag="acc")
            first = True
            pending = None
            for t in range(NTILE):
                if h == 0:
                    for ch in _STAGE_AT.get(t, ()):
                        stage_chunk(ch)
                jts = list(range(t * JT_PER_TILE,
                                 min((t + 1) * JT_PER_TILE, NT_D)))
                fd = 512 * len(jts)
                pm = ps_main.tile([P, 1536], F32, tag="pm")
                e = e_pool.tile([P, 1536], F32R, tag="e")
                for cix, jt in enumerate(jts):
                    nc.tensor.matmul(pm[:, cix * 512:(cix + 1) * 512],
                                     dataT[:, jt * P:(jt + 1) * P], xsl,
                                     start=True, stop=True)
                nc.scalar.activation(e[:, 0:fd], pm[:, 0:fd],
                                     mybir.ActivationFunctionType.Exp,
                                     scale=TWO_C)
                # w for freshly staged chunks, after this tile's exp so it
                # never blocks the ACT FIFO ahead of an exp
                if h == 0:
                    for ch in _STAGE_AT.get(t, ()):
                        s0 = ch * TPC
                        nc.scalar.activation(
                            wj[:, s0:s0 + TPC], dnsq[:, s0:s0 + TPC],
                            mybir.ActivationFunctionType.Exp, scale=-C)
                # matvecs lag one tile so a late w never head-of-line
                # blocks the PE FIFO in front of the next tile's GEMMs
                for (ee, ejts) in ([pending] if pending else []):
                    for cix, jt in enumerate(ejts):
                        nc.tensor.matmul(acc[:], wj[:, jt:jt + 1],
                                         ee[:, cix * 512:(cix + 1) * 512],
                                         start=first,
                                         stop=(jt == NT_D - 1),
                                         skip_group_check=True)
                        first = False
                pending = (e, ejts0 := jts)
            ee, ejts = pending
            for cix, jt in enumerate(ejts):
                nc.tensor.matmul(acc[:], wj[:, jt:jt + 1],
                                 ee[:, cix * 512:(cix + 1) * 512],
                                 start=first, stop=(jt == NT_D - 1),
                                 skip_group_check=True)
                first = False
            if h == 0:
                # x-norm factor exf = exp(-C||x_i||^2 - lnM), off the ramp's
                # critical path (only needed by the pass epilogues)
                nc.vector.tensor_mul(xsqT[:], xT[:], xT[:])
                for c2 in range(2):
                    pmx = ps_x.tile([P, 512], F32, tag="px")
                    sl = slice(c2 * 512, (c2 + 1) * 512)
                    nc.tensor.matmul(pmx[0:1, :], ones_r[:], xsqT[:, sl],
                                     start=True, stop=True)
                    nc.scalar.activation(exf[:, sl], pmx[0:1, :],
                                         mybir.ActivationFunctionType.Exp,
                                         bias=nlm_bias[:], scale=-C)
            hsl = slice(h * 512, (h + 1) * 512)
            nc.vector.tensor_mul(dens[:, hsl], acc[:], exf[:, hsl])
            # undo the x row permutation for this half: col r*128+p -> row
            # 8p+r; half h covers r in [4h, 4h+4)
            nc.vector.tensor_copy(
                dens_o[:].rearrange("o (p r) -> o p r", r=8)[:, :, 4 * h:4 * h + 4],
                dens[:, hsl].rearrange("o (r p) -> o p r", p=P))

        nc.sync.dma_start(o_d.ap(), dens_o[:])

    nc.compile()
    return nc


def kernel(x, data):
    global _CACHED_NC
    x = np.ascontiguousarray(np.asarray(x, dtype=np.float32))
    data = np.ascontiguousarray(np.asarray(data, dtype=np.float32))
    assert x.shape == (N, D) and data.shape == (M, D)

    if _CACHED_NC is None:
        _CACHED_NC = _build()
    nc = _CACHED_NC

    in_maps = [
        {"x": x[c * NS:(c + 1) * NS], "data": data} for c in range(NCORES)
    ]
    res = run_bass_kernel_spmd(nc, in_maps, list(range(NCORES)))
    dens = np.concatenate(
        [np.asarray(res.results[c]["out"]).reshape(NS) for c in range(NCORES)]
    )
    return dens.reshape(N, 1).astype(np.float32)


if __name__ == "__main__":
    rng = np.random.default_rng(0)
    x = rng.standard_normal((N, D), dtype=np.float32)
    data = rng.standard_normal((M, D), dtype=np.float32)
    out = kernel(x, data)
    print("kernel out", out.shape, out[:4, 0])
